# revision 25
# baseline (speedup 1.0000x reference)
"""TRN2 Bass kernel for nn_AlignS (GraphSAGE + soft pooling, 64 graphs).

Strategy: graph-level data parallelism (8 graphs per NeuronCore).
The irregular segment-sums (mean-aggregate A@h and pooling A@S) are turned
into dense TensorEngine matmuls against a host-prebuilt adjacency-count
matrix A^T (bf16-exact small integers) streamed through the PE; 1/deg is
folded into the z assembly as a per-partition scalar. Host prep is
layout-only (bincount of the edge list + transposes); all FLOPs of the
reference run on device:

  per graph g (n=1024 nodes, D=128, K=64, C=8 node chunks of 128):
    hS,hN   = h @ [W_self|W_neigh]            (PE, weights = h^T chunks)
    msum    = hN^T-contract A^T               (PE, K-major out [64,1024])
    z       = msum^T/deg + hS + b             (PE transposes + DVE stt)
    S       = softmax_k(relu(z))              (ACT exp, DVE reduce/recip)
    AS      = A @ S                           (PE, same A tile reused)
    blocks  = S^T @ AS                        (PE)
    entropy terms: sum(S*relu z), sum(log rowsum), sum(S^2)  (DVE accums)

Output per core: 8 k x k blocks + 3 scalar partials; host places blocks on
the block-diagonal and combines the scalar partials.
"""

import numpy as np

B = 64
N = 1024
D = 128
K = 64
E = 16384
NCORES = 8
G = B // NCORES          # graphs per core
C = N // 128             # 128-node chunks per graph
Q = 4                    # chunks per softmax pipeline stage

USE_BF16 = True
TRACE = False
LAST_EXEC_NS = None

_COMPILED = {}


def _adt_np():
    if USE_BF16:
        import ml_dtypes
        return ml_dtypes.bfloat16
    return np.float32


def _build():
    import concourse.bacc as bacc
    import concourse.mybir as mybir
    import concourse.tile as tile

    f32 = mybir.dt.float32
    adt = mybir.dt.bfloat16 if USE_BF16 else mybir.dt.float32
    Alu = mybir.AluOpType
    Act = mybir.ActivationFunctionType

    nc = bacc.Bacc("TRN2", target_bir_lowering=False, debug=False,
                   num_devices=NCORES)

    hT_d = nc.dram_tensor("hT", [128, G * N], adt, kind="ExternalInput").ap()
    wcat_d = nc.dram_tensor("wcat", [128, 128], adt, kind="ExternalInput").ap()
    bb_d = nc.dram_tensor("bb", [128, 4 * K], f32, kind="ExternalInput").ap()
    ident_d = nc.dram_tensor("ident", [64, 64], f32, kind="ExternalInput").ap()
    identb_d = nc.dram_tensor("identb", [64, 64], adt, kind="ExternalInput").ap()
    ats_d = nc.dram_tensor("ats", [G, 128, C * N], adt, kind="ExternalInput").ap()
    idg_d = nc.dram_tensor("idg", [128, G * C], f32, kind="ExternalInput").ap()

    blocks_d = nc.dram_tensor("blocks", [64, G * 64], f32, kind="ExternalOutput").ap()
    scal_d = nc.dram_tensor("scal", [1, 4], f32, kind="ExternalOutput").ap()

    with tile.TileContext(nc) as tc:
        with (
            tc.tile_pool(name="persist", bufs=1) as pp,
            tc.tile_pool(name="gpool", bufs=3) as gp,
            tc.tile_pool(name="apool", bufs=4) as ap_,
            tc.tile_pool(name="psum2", bufs=2, space="PSUM") as qp,
            tc.tile_pool(name="psum1", bufs=1, space="PSUM") as q1,
        ):
            wcat = pp.tile([128, 128], adt)
            bb = pp.tile([128, 4 * K], f32)
            ident = pp.tile([64, 64], f32)
            identb = pp.tile([64, 64], adt)
            idg = pp.tile([128, G * C], f32)
            ones128 = pp.tile([128, 1], f32)
            ent_strip = pp.tile([128, G], f32)
            sq_strip = pp.tile([128, G], f32)
            se_all = pp.tile([128, G * C], f32)
            lse_strip = pp.tile([128, G * C], f32)
            blocks_s = pp.tile([64, G * 64], f32)
            pack3 = pp.tile([128, 3], f32)
            scal_s = pp.tile([1, 4], f32)

            nc.sync.dma_start(wcat[:], wcat_d[:])
            nc.sync.dma_start(bb[:], bb_d[:])
            nc.sync.dma_start(ident[:], ident_d[:])
            nc.sync.dma_start(identb[:], identb_d[:])
            nc.sync.dma_start(idg[:], idg_d[:])
            nc.vector.memset(ones128[:], 1.0)
            nc.vector.memset(scal_s[:], 0.0)

            def project(g):
                # load hT slice + A tile, compute hS/hN node-rows for graph g
                hTg = gp.tile([128, N], adt, tag="hTg")
                atile = ap_.tile([128, C * N], adt, tag="atile")
                nc.sync.dma_start(hTg[:], hT_d[:, g * N:(g + 1) * N])
                for q4 in range(4):
                    nc.sync.dma_start(atile[:, q4 * 2 * N:(q4 + 1) * 2 * N],
                                      ats_d[g][:, q4 * 2 * N:(q4 + 1) * 2 * N])
                hs_rows = gp.tile([128, C * K], f32, tag="hs")
                hn_w = gp.tile([128, C * K], adt, tag="hn")
                for c4 in range(C // 4):
                    ps = q1.tile([128, 512], f32, tag="hsn", space="PSUM")
                    for j in range(4):
                        c = c4 * 4 + j
                        nc.tensor.matmul(ps[:, j * 128:(j + 1) * 128],
                                         lhsT=hTg[:, c * 128:(c + 1) * 128],
                                         rhs=wcat[:], start=True, stop=True)
                    ps3 = ps[:].rearrange("p (j k) -> p j k", k=K)
                    col = c4 * 4 * K
                    nc.vector.scalar_tensor_tensor(
                        out=hs_rows[:, col:col + 4 * K].rearrange(
                            "p (j k) -> p j k", k=K),
                        in0=ps3[:, 0:8:2], scalar=1.0,
                        in1=bb[:].rearrange("p (j k) -> p j k", k=K),
                        op0=Alu.mult, op1=Alu.add)
                    nc.scalar.activation(
                        hn_w[:, col:col + 4 * K].rearrange(
                            "p (j k) -> p j k", k=K),
                        ps3[:, 1:8:2], Act.Copy)
                return atile, hs_rows, hn_w

            def phase1(atile, hn_w):
                # msum[k, v] = sum_u hN[u, k] * A^T[u, v]
                ps_m0 = qp.tile([64, 512], f32, tag="accA", space="PSUM")
                ps_m1 = qp.tile([64, 512], f32, tag="accB", space="PSUM")
                for c in range(C):
                    lhs = hn_w[:, c * K:(c + 1) * K]
                    base = c * N
                    nc.tensor.matmul(ps_m0[:], lhsT=lhs,
                                     rhs=atile[:, base:base + 512],
                                     start=(c == 0), stop=(c == C - 1))
                    nc.tensor.matmul(ps_m1[:], lhsT=lhs,
                                     rhs=atile[:, base + 512:base + N],
                                     start=(c == 0), stop=(c == C - 1))
                msum_s = gp.tile([64, N], adt, tag="msum")
                nc.scalar.activation(msum_s[:, 0:512], ps_m0[:], Act.Copy)
                nc.scalar.activation(msum_s[:, 512:N], ps_m1[:], Act.Copy)
                return msum_s

            cur = project(0)
            cur_msum = phase1(cur[0], cur[2])
            nxt = project(1)
            for g in range(G):
                atile, hs_rows, hn_w = cur
                msum_s = cur_msum

                # per-quarter pipeline: transpose+z -> exp -> softmax -> S cast
                zraw = gp.tile([128, C * K], f32, tag="zraw")
                expm = gp.tile([128, C * K], f32, tag="expm")
                rcp8 = gp.tile([128, C], f32, tag="rcp8")
                Sf = gp.tile([128, C * K], f32, tag="Sf")
                Sb = gp.tile([128, C * K], adt, tag="Sb")
                for c2 in range(C // 2):
                    ps_t = qp.tile([128, 128], adt, tag="tr", space="PSUM")
                    for j in range(2):
                        c = c2 * 2 + j
                        nc.tensor.transpose(ps_t[:, j * K:(j + 1) * K],
                                            in_=msum_s[:, c * 128:(c + 1) * 128],
                                            identity=identb[:])
                    sl = slice(c2 * 2 * K, (c2 + 1) * 2 * K)
                    nc.vector.tensor_tensor(
                        out=zraw[:, sl].rearrange("p (c k) -> p c k", k=K),
                        in0=ps_t[:].rearrange("p (c k) -> p c k", k=K),
                        in1=idg[:, g * C + c2 * 2:g * C + c2 * 2 + 2]
                            .to_broadcast([128, 2, K]),
                        op=Alu.mult)
                    nc.vector.tensor_tensor(
                        out=zraw[:, sl], in0=zraw[:, sl], in1=hs_rows[:, sl],
                        op=Alu.add)
                    if (c2 * 2) % Q == Q - 2:
                        q0 = (c2 * 2 // Q) * Q * K
                        qc = (c2 * 2 // Q) * Q
                        sl = slice(q0, q0 + Q * K)
                        nc.scalar.activation(expm[:, sl], zraw[:, sl], Act.Exp)
                        nc.vector.tensor_scalar_max(expm[:, sl], expm[:, sl], 1.0)
                        nc.vector.tensor_reduce(
                            out=se_all[:, g * C + qc:g * C + qc + Q],
                            in_=expm[:, sl].rearrange("p (c k) -> p c k", k=K),
                            op=Alu.add, axis=mybir.AxisListType.X)
                        nc.vector.reciprocal(
                            rcp8[:, qc:qc + Q],
                            se_all[:, g * C + qc:g * C + qc + Q])
                        nc.vector.tensor_tensor(
                            out=Sf[:, sl].rearrange("p (c k) -> p c k", k=K),
                            in0=expm[:, sl].rearrange("p (c k) -> p c k", k=K),
                            in1=rcp8[:, qc:qc + Q].to_broadcast([128, Q, K]),
                            op=Alu.mult)
                        nc.vector.tensor_copy(Sb[:, sl], Sf[:, sl])

                if g + 1 < G:
                    cur = nxt
                    cur_msum = phase1(cur[0], cur[2])

                # entropy + l2 partials (relu folded into the entropy op)
                nc.vector.scalar_tensor_tensor(
                    out=zraw[:], in0=zraw[:], scalar=0.0, in1=Sf[:],
                    op0=Alu.max, op1=Alu.mult,
                    accum_out=ent_strip[:, g:g + 1])
                nc.vector.scalar_tensor_tensor(
                    out=expm[:], in0=Sf[:], scalar=1.0, in1=Sf[:],
                    op0=Alu.mult, op1=Alu.mult,
                    accum_out=sq_strip[:, g:g + 1])

                # phase 2: AS[m, v] = sum_u S[u, m] * A^T[u, v]
                ps_a0 = qp.tile([64, 512], f32, tag="accA", space="PSUM")
                ps_a1 = qp.tile([64, 512], f32, tag="accB", space="PSUM")
                for c in range(C):
                    lhs = Sb[:, c * K:(c + 1) * K]
                    base = c * N
                    nc.tensor.matmul(ps_a0[:], lhsT=lhs,
                                     rhs=atile[:, base:base + 512],
                                     start=(c == 0), stop=(c == C - 1))
                    nc.tensor.matmul(ps_a1[:], lhsT=lhs,
                                     rhs=atile[:, base + 512:base + N],
                                     start=(c == 0), stop=(c == C - 1))
                as0_s = gp.tile([64, N], adt, tag="as0")
                nc.scalar.activation(as0_s[:, 0:512], ps_a0[:], Act.Copy)
                nc.scalar.activation(as0_s[:, 512:N], ps_a1[:], Act.Copy)

                as0_rows = gp.tile([128, C * K], adt, tag="as0r")
                for c2 in range(C // 2):
                    ps_t2 = qp.tile([128, 128], adt, tag="tr", space="PSUM")
                    for j in range(2):
                        c = c2 * 2 + j
                        nc.tensor.transpose(ps_t2[:, j * K:(j + 1) * K],
                                            in_=as0_s[:, c * 128:(c + 1) * 128],
                                            identity=identb[:])
                    nc.scalar.activation(
                        as0_rows[:, c2 * 128:(c2 + 1) * 128], ps_t2[:],
                        Act.Copy)

                ps_blk = q1.tile([64, 64], f32, tag="blk", space="PSUM")
                for c in range(C):
                    nc.tensor.matmul(ps_blk[:],
                                     lhsT=Sb[:, c * K:(c + 1) * K],
                                     rhs=as0_rows[:, c * K:(c + 1) * K],
                                     start=(c == 0), stop=(c == C - 1))
                nc.vector.tensor_copy(blocks_s[:, g * 64:(g + 1) * 64], ps_blk[:])
                nc.sync.dma_start(blocks_d[:, g * 64:(g + 1) * 64],
                                  blocks_s[:, g * 64:(g + 1) * 64])
                if g + 2 < G:
                    nxt = project(g + 2)

            # finalize scalar partials
            nc.scalar.activation(lse_strip[:], se_all[:], Act.Ln)
            nc.vector.tensor_reduce(out=pack3[:, 0:1], in_=ent_strip[:],
                                    op=Alu.add, axis=mybir.AxisListType.X)
            nc.vector.tensor_reduce(out=pack3[:, 1:2], in_=lse_strip[:],
                                    op=Alu.add, axis=mybir.AxisListType.X)
            nc.vector.tensor_reduce(out=pack3[:, 2:3], in_=sq_strip[:],
                                    op=Alu.add, axis=mybir.AxisListType.X)
            ps_sc = q1.tile([1, 4], f32, tag="blk", space="PSUM")
            nc.tensor.matmul(ps_sc[:, 0:3], lhsT=ones128[:], rhs=pack3[:],
                             start=True, stop=True)
            nc.vector.tensor_copy(scal_s[:, 0:3], ps_sc[:, 0:3])

            nc.sync.dma_start(scal_d[:], scal_s[:])

    nc.compile()
    return nc


def _get_compiled():
    if "nc" not in _COMPILED:
        _COMPILED["nc"] = _build()
    return _COMPILED["nc"]


def _host_prep(h, W_self, W_neigh, b, src, dst):
    adt = _adt_np()
    h = np.asarray(h, np.float32)
    W_self = np.asarray(W_self, np.float32)
    W_neigh = np.asarray(W_neigh, np.float32)
    b = np.asarray(b, np.float32)
    src = np.asarray(src)
    dst = np.asarray(dst)

    wcat = np.ascontiguousarray(
        np.concatenate([W_self, W_neigh], axis=1)).astype(adt)
    bbt = np.ascontiguousarray(np.tile(b[None, :], (128, 4)))
    ident = np.eye(64, dtype=np.float32)

    in_maps = []
    for core in range(NCORES):
        rows = slice(core * G * N, (core + 1) * G * N)
        hT = np.ascontiguousarray(h[rows].T).astype(adt)
        ats = np.empty((G, 128, C * N), adt)
        idg = np.empty((128, G * C), np.float32)
        for g in range(G):
            gb = core * G + g
            s = src[gb].astype(np.int64)
            d = dst[gb].astype(np.int64)
            cnt = np.bincount(d * N + s, minlength=N * N).astype(np.float32)
            A = cnt.reshape(N, N)                       # A[v, u] = #edges u->v
            deg = np.bincount(d, minlength=N).astype(np.float32)
            dc = np.maximum(deg, 1.0)
            ats[g] = (A.T.reshape(C, 128, N).transpose(1, 0, 2)
                      .reshape(128, C * N).astype(adt))
            idg[:, g * C:(g + 1) * C] = (1.0 / dc).reshape(C, 128).T
        in_maps.append({"hT": hT, "wcat": wcat, "bb": bbt, "ident": ident,
                        "identb": ident.astype(adt), "ats": ats, "idg": idg})
    return in_maps


def kernel(h, W_self, W_neigh, b, src, dst):
    global LAST_EXEC_NS
    from concourse.bass_utils import run_bass_kernel_spmd

    nc = _get_compiled()
    in_maps = _host_prep(h, W_self, W_neigh, b, src, dst)
    res = run_bass_kernel_spmd(nc, in_maps, core_ids=list(range(NCORES)),
                               trace=TRACE)
    LAST_EXEC_NS = res.exec_time_ns

    adj = np.zeros((B * K, B * K), np.float32)
    ent_sum = 0.0
    lse_sum = 0.0
    sq_sum = 0.0
    for core in range(NCORES):
        blk = res.results[core]["blocks"]
        sc = res.results[core]["scal"][0]
        for g in range(G):
            gb = core * G + g
            adj[gb * K:(gb + 1) * K, gb * K:(gb + 1) * K] = \
                blk[:, g * 64:(g + 1) * 64]
        ent_sum += float(sc[0])
        lse_sum += float(sc[1])
        sq_sum += float(sc[2])

    entropy = -(ent_sum - lse_sum) / (B * N)
    loss = np.float32(entropy + np.sqrt(sq_sum))
    return adj, loss


# revision 26
# speedup vs baseline: 1.3085x; 1.3085x over previous
"""TRN2 Bass kernel for nn_AlignS (GraphSAGE + soft pooling, 64 graphs).

Strategy: graph-level data parallelism (8 graphs per NeuronCore).
The irregular segment-sums (mean-aggregate A@h and pooling A@S) are turned
into dense TensorEngine matmuls against a host-prebuilt adjacency-count
matrix A^T (bf16-exact small integers) streamed through the PE; 1/deg is
folded into the z assembly as a per-partition scalar. Host prep is
layout-only (bincount of the edge list + transposes); all FLOPs of the
reference run on device:

  per graph g (n=1024 nodes, D=128, K=64, C=8 node chunks of 128):
    hS,hN   = h @ [W_self|W_neigh]            (PE, weights = h^T chunks)
    msum    = hN^T-contract A^T               (PE, K-major out [64,1024])
    z       = msum^T/deg + hS + b             (PE transposes + DVE stt)
    S       = softmax_k(relu(z))              (ACT exp, DVE reduce/recip)
    AS      = A @ S                           (PE, same A tile reused)
    blocks  = S^T @ AS                        (PE)
    entropy terms: sum(S*relu z), sum(log rowsum), sum(S^2)  (DVE accums)

Output per core: 8 k x k blocks + 3 scalar partials; host places blocks on
the block-diagonal and combines the scalar partials.
"""

import numpy as np

B = 64
N = 1024
D = 128
K = 64
E = 16384
NCORES = 8
G = B // NCORES          # graphs per core
C = N // 128             # 128-node chunks per graph
Q = 2                    # chunks per softmax pipeline stage

USE_BF16 = True
TRACE = False
LAST_EXEC_NS = None

_COMPILED = {}


def _adt_np():
    if USE_BF16:
        import ml_dtypes
        return ml_dtypes.bfloat16
    return np.float32


def _build():
    import concourse.bacc as bacc
    import concourse.mybir as mybir
    import concourse.tile as tile

    f32 = mybir.dt.float32
    adt = mybir.dt.bfloat16 if USE_BF16 else mybir.dt.float32
    Alu = mybir.AluOpType
    Act = mybir.ActivationFunctionType

    nc = bacc.Bacc("TRN2", target_bir_lowering=False, debug=False,
                   num_devices=NCORES)

    hT_d = nc.dram_tensor("hT", [128, G * N], adt, kind="ExternalInput").ap()
    wcat_d = nc.dram_tensor("wcat", [128, 128], adt, kind="ExternalInput").ap()
    bb_d = nc.dram_tensor("bb", [128, 4 * K], f32, kind="ExternalInput").ap()
    ident_d = nc.dram_tensor("ident", [64, 64], f32, kind="ExternalInput").ap()
    identb_d = nc.dram_tensor("identb", [64, 64], adt, kind="ExternalInput").ap()
    ats_d = nc.dram_tensor("ats", [G, 128, C * N], adt, kind="ExternalInput").ap()
    idg_d = nc.dram_tensor("idg", [128, G * C], f32, kind="ExternalInput").ap()

    blocks_d = nc.dram_tensor("blocks", [64, G * 64], f32, kind="ExternalOutput").ap()
    scal_d = nc.dram_tensor("scal", [1, 4], f32, kind="ExternalOutput").ap()

    with tile.TileContext(nc) as tc:
        with (
            tc.tile_pool(name="persist", bufs=1) as pp,
            tc.tile_pool(name="gpool", bufs=3) as gp,
            tc.tile_pool(name="apool", bufs=4) as ap_,
            tc.tile_pool(name="psum2", bufs=2, space="PSUM") as qp,
            tc.tile_pool(name="psum1", bufs=1, space="PSUM") as q1,
        ):
            wcat = pp.tile([128, 128], adt)
            bb = pp.tile([128, 4 * K], f32)
            ident = pp.tile([64, 64], f32)
            identb = pp.tile([64, 64], adt)
            idg = pp.tile([128, G * C], f32)
            ones128 = pp.tile([128, 1], f32)
            ent_strip = pp.tile([128, G], f32)
            sq_strip = pp.tile([128, G], f32)
            se_all = pp.tile([128, G * C], f32)
            lse_strip = pp.tile([128, G * C], f32)
            blocks_s = pp.tile([64, G * 64], f32)
            pack3 = pp.tile([128, 3], f32)
            scal_s = pp.tile([1, 4], f32)

            nc.sync.dma_start(wcat[:], wcat_d[:])
            nc.sync.dma_start(bb[:], bb_d[:])
            nc.sync.dma_start(ident[:], ident_d[:])
            nc.sync.dma_start(identb[:], identb_d[:])
            nc.sync.dma_start(idg[:], idg_d[:])
            nc.vector.memset(ones128[:], 1.0)
            nc.vector.memset(scal_s[:], 0.0)

            def project(g):
                # load hT slice + A tile, compute hS/hN node-rows for graph g
                hTg = gp.tile([128, N], adt, tag="hTg")
                atile = ap_.tile([128, C * N], adt, tag="atile")
                nc.sync.dma_start(hTg[:], hT_d[:, g * N:(g + 1) * N])
                for q4 in range(4):
                    nc.sync.dma_start(atile[:, q4 * 2 * N:(q4 + 1) * 2 * N],
                                      ats_d[g][:, q4 * 2 * N:(q4 + 1) * 2 * N])
                hs_rows = gp.tile([128, C * K], f32, tag="hs")
                hn_w = gp.tile([128, C * K], adt, tag="hn")
                for c4 in range(C // 4):
                    ps = q1.tile([128, 512], f32, tag="hsn", space="PSUM")
                    for j in range(4):
                        c = c4 * 4 + j
                        nc.tensor.matmul(ps[:, j * 128:(j + 1) * 128],
                                         lhsT=hTg[:, c * 128:(c + 1) * 128],
                                         rhs=wcat[:], start=True, stop=True)
                    ps3 = ps[:].rearrange("p (j k) -> p j k", k=K)
                    col = c4 * 4 * K
                    nc.vector.scalar_tensor_tensor(
                        out=hs_rows[:, col:col + 4 * K].rearrange(
                            "p (j k) -> p j k", k=K),
                        in0=ps3[:, 0:8:2], scalar=1.0,
                        in1=bb[:].rearrange("p (j k) -> p j k", k=K),
                        op0=Alu.mult, op1=Alu.add)
                    nc.scalar.activation(
                        hn_w[:, col:col + 4 * K].rearrange(
                            "p (j k) -> p j k", k=K),
                        ps3[:, 1:8:2], Act.Copy)
                return atile, hs_rows, hn_w

            def phase1(atile, hn_w):
                # msum[k, v] = sum_u hN[u, k] * A^T[u, v]
                ps_m0 = qp.tile([64, 512], f32, tag="accA", space="PSUM")
                ps_m1 = qp.tile([64, 512], f32, tag="accB", space="PSUM")
                for c in range(C):
                    lhs = hn_w[:, c * K:(c + 1) * K]
                    base = c * N
                    nc.tensor.matmul(ps_m0[:], lhsT=lhs,
                                     rhs=atile[:, base:base + 512],
                                     start=(c == 0), stop=(c == C - 1))
                    nc.tensor.matmul(ps_m1[:], lhsT=lhs,
                                     rhs=atile[:, base + 512:base + N],
                                     start=(c == 0), stop=(c == C - 1))
                msum_s = gp.tile([64, N], adt, tag="msum")
                nc.scalar.activation(msum_s[:, 0:512], ps_m0[:], Act.Copy)
                nc.scalar.activation(msum_s[:, 512:N], ps_m1[:], Act.Copy)
                return msum_s

            cur = project(0)
            cur_msum = phase1(cur[0], cur[2])
            nxt = project(1)
            for g in range(G):
                atile, hs_rows, hn_w = cur
                msum_s = cur_msum

                # per-quarter pipeline: transpose+z -> exp -> softmax -> S cast
                zraw = gp.tile([128, C * K], f32, tag="zraw")
                expm = gp.tile([128, C * K], f32, tag="expm")
                rcp8 = gp.tile([128, C], f32, tag="rcp8")
                Sf = gp.tile([128, C * K], f32, tag="Sf")
                Sb = gp.tile([128, C * K], adt, tag="Sb")
                for c2 in range(C // 2):
                    ps_t = qp.tile([128, 128], adt, tag="tr", space="PSUM")
                    for j in range(2):
                        c = c2 * 2 + j
                        nc.tensor.transpose(ps_t[:, j * K:(j + 1) * K],
                                            in_=msum_s[:, c * 128:(c + 1) * 128],
                                            identity=identb[:])
                    sl = slice(c2 * 2 * K, (c2 + 1) * 2 * K)
                    nc.vector.tensor_tensor(
                        out=zraw[:, sl].rearrange("p (c k) -> p c k", k=K),
                        in0=ps_t[:].rearrange("p (c k) -> p c k", k=K),
                        in1=idg[:, g * C + c2 * 2:g * C + c2 * 2 + 2]
                            .to_broadcast([128, 2, K]),
                        op=Alu.mult)
                    nc.vector.tensor_tensor(
                        out=zraw[:, sl], in0=zraw[:, sl], in1=hs_rows[:, sl],
                        op=Alu.add)
                    if (c2 * 2) % Q == Q - 2:
                        q0 = (c2 * 2 // Q) * Q * K
                        qc = (c2 * 2 // Q) * Q
                        sl = slice(q0, q0 + Q * K)
                        # (Q == 2: one softmax stage per transpose pair)
                        nc.scalar.activation(expm[:, sl], zraw[:, sl], Act.Exp)
                        nc.vector.tensor_scalar_max(expm[:, sl], expm[:, sl], 1.0)
                        nc.vector.tensor_reduce(
                            out=se_all[:, g * C + qc:g * C + qc + Q],
                            in_=expm[:, sl].rearrange("p (c k) -> p c k", k=K),
                            op=Alu.add, axis=mybir.AxisListType.X)
                        nc.vector.reciprocal(
                            rcp8[:, qc:qc + Q],
                            se_all[:, g * C + qc:g * C + qc + Q])
                        nc.vector.tensor_tensor(
                            out=Sf[:, sl].rearrange("p (c k) -> p c k", k=K),
                            in0=expm[:, sl].rearrange("p (c k) -> p c k", k=K),
                            in1=rcp8[:, qc:qc + Q].to_broadcast([128, Q, K]),
                            op=Alu.mult)
                        nc.vector.tensor_copy(Sb[:, sl], Sf[:, sl])

                if g + 1 < G:
                    cur = nxt
                    cur_msum = phase1(cur[0], cur[2])

                # entropy + l2 partials (relu folded into the entropy op)
                nc.vector.scalar_tensor_tensor(
                    out=zraw[:], in0=zraw[:], scalar=0.0, in1=Sf[:],
                    op0=Alu.max, op1=Alu.mult,
                    accum_out=ent_strip[:, g:g + 1])
                nc.vector.scalar_tensor_tensor(
                    out=expm[:], in0=Sf[:], scalar=1.0, in1=Sf[:],
                    op0=Alu.mult, op1=Alu.mult,
                    accum_out=sq_strip[:, g:g + 1])

                # phase 2: AS[m, v] = sum_u S[u, m] * A^T[u, v]
                ps_a0 = qp.tile([64, 512], f32, tag="accA", space="PSUM")
                ps_a1 = qp.tile([64, 512], f32, tag="accB", space="PSUM")
                for c in range(C):
                    lhs = Sb[:, c * K:(c + 1) * K]
                    base = c * N
                    nc.tensor.matmul(ps_a0[:], lhsT=lhs,
                                     rhs=atile[:, base:base + 512],
                                     start=(c == 0), stop=(c == C - 1))
                    nc.tensor.matmul(ps_a1[:], lhsT=lhs,
                                     rhs=atile[:, base + 512:base + N],
                                     start=(c == 0), stop=(c == C - 1))
                as0_s = gp.tile([64, N], adt, tag="as0")
                nc.scalar.activation(as0_s[:, 0:512], ps_a0[:], Act.Copy)
                nc.scalar.activation(as0_s[:, 512:N], ps_a1[:], Act.Copy)

                as0_rows = gp.tile([128, C * K], adt, tag="as0r")
                for c2 in range(C // 2):
                    ps_t2 = qp.tile([128, 128], adt, tag="tr", space="PSUM")
                    for j in range(2):
                        c = c2 * 2 + j
                        nc.tensor.transpose(ps_t2[:, j * K:(j + 1) * K],
                                            in_=as0_s[:, c * 128:(c + 1) * 128],
                                            identity=identb[:])
                    nc.scalar.activation(
                        as0_rows[:, c2 * 128:(c2 + 1) * 128], ps_t2[:],
                        Act.Copy)

                ps_blk = q1.tile([64, 64], f32, tag="blk", space="PSUM")
                for c in range(C):
                    nc.tensor.matmul(ps_blk[:],
                                     lhsT=Sb[:, c * K:(c + 1) * K],
                                     rhs=as0_rows[:, c * K:(c + 1) * K],
                                     start=(c == 0), stop=(c == C - 1))
                nc.vector.tensor_copy(blocks_s[:, g * 64:(g + 1) * 64], ps_blk[:])
                nc.sync.dma_start(blocks_d[:, g * 64:(g + 1) * 64],
                                  blocks_s[:, g * 64:(g + 1) * 64])
                if g + 2 < G:
                    nxt = project(g + 2)

            # finalize scalar partials
            nc.scalar.activation(lse_strip[:], se_all[:], Act.Ln)
            nc.vector.tensor_reduce(out=pack3[:, 0:1], in_=ent_strip[:],
                                    op=Alu.add, axis=mybir.AxisListType.X)
            nc.vector.tensor_reduce(out=pack3[:, 1:2], in_=lse_strip[:],
                                    op=Alu.add, axis=mybir.AxisListType.X)
            nc.vector.tensor_reduce(out=pack3[:, 2:3], in_=sq_strip[:],
                                    op=Alu.add, axis=mybir.AxisListType.X)
            ps_sc = q1.tile([1, 4], f32, tag="blk", space="PSUM")
            nc.tensor.matmul(ps_sc[:, 0:3], lhsT=ones128[:], rhs=pack3[:],
                             start=True, stop=True)
            nc.vector.tensor_copy(scal_s[:, 0:3], ps_sc[:, 0:3])

            nc.sync.dma_start(scal_d[:], scal_s[:])

    nc.compile()
    return nc


def _get_compiled():
    if "nc" not in _COMPILED:
        _COMPILED["nc"] = _build()
    return _COMPILED["nc"]


def _host_prep(h, W_self, W_neigh, b, src, dst):
    adt = _adt_np()
    h = np.asarray(h, np.float32)
    W_self = np.asarray(W_self, np.float32)
    W_neigh = np.asarray(W_neigh, np.float32)
    b = np.asarray(b, np.float32)
    src = np.asarray(src)
    dst = np.asarray(dst)

    wcat = np.ascontiguousarray(
        np.concatenate([W_self, W_neigh], axis=1)).astype(adt)
    bbt = np.ascontiguousarray(np.tile(b[None, :], (128, 4)))
    ident = np.eye(64, dtype=np.float32)

    in_maps = []
    for core in range(NCORES):
        rows = slice(core * G * N, (core + 1) * G * N)
        hT = np.ascontiguousarray(h[rows].T).astype(adt)
        ats = np.empty((G, 128, C * N), adt)
        idg = np.empty((128, G * C), np.float32)
        for g in range(G):
            gb = core * G + g
            s = src[gb].astype(np.int64)
            d = dst[gb].astype(np.int64)
            cnt = np.bincount(d * N + s, minlength=N * N).astype(np.float32)
            A = cnt.reshape(N, N)                       # A[v, u] = #edges u->v
            deg = np.bincount(d, minlength=N).astype(np.float32)
            dc = np.maximum(deg, 1.0)
            ats[g] = (A.T.reshape(C, 128, N).transpose(1, 0, 2)
                      .reshape(128, C * N).astype(adt))
            idg[:, g * C:(g + 1) * C] = (1.0 / dc).reshape(C, 128).T
        in_maps.append({"hT": hT, "wcat": wcat, "bb": bbt, "ident": ident,
                        "identb": ident.astype(adt), "ats": ats, "idg": idg})
    return in_maps


def kernel(h, W_self, W_neigh, b, src, dst):
    global LAST_EXEC_NS
    from concourse.bass_utils import run_bass_kernel_spmd

    nc = _get_compiled()
    in_maps = _host_prep(h, W_self, W_neigh, b, src, dst)
    res = run_bass_kernel_spmd(nc, in_maps, core_ids=list(range(NCORES)),
                               trace=TRACE)
    LAST_EXEC_NS = res.exec_time_ns

    adj = np.zeros((B * K, B * K), np.float32)
    ent_sum = 0.0
    lse_sum = 0.0
    sq_sum = 0.0
    for core in range(NCORES):
        blk = res.results[core]["blocks"]
        sc = res.results[core]["scal"][0]
        for g in range(G):
            gb = core * G + g
            adj[gb * K:(gb + 1) * K, gb * K:(gb + 1) * K] = \
                blk[:, g * 64:(g + 1) * 64]
        ent_sum += float(sc[0])
        lse_sum += float(sc[1])
        sq_sum += float(sc[2])

    entropy = -(ent_sum - lse_sum) / (B * N)
    loss = np.float32(entropy + np.sqrt(sq_sum))
    return adj, loss
